# revision 29
# baseline (speedup 1.0000x reference)
"""Trainium2 Bass kernel for CHSLoss (top-k masked MSE), 8-core data parallel.

Math (per batch row, n = H*W elements, k = int(n * 0.1 * process)):
    gt   = 8x8 sum-pool of gt_density
    d_i  = map_i - gt,  err_i = |d_i|
    mask_i = err_i >= (k-th largest of err_i)
    loss += sum(d_i^2) + sum(mask_i * ((d_i - w*d_j)^2 - d_i^2))   (j != i)

Device strategy per core (2 batches/core):
  - row-pool via SWDGE accumulate-DMA (8 passes, CCE add), col-pool via a
    3D free-dim reduce -> pooled gt in SBUF; all elementwise stages on
    DVE/ACT.
  - threshold search: per-(map,batch) Gaussian-stats initial bracket
    (t0 = mu + a*sigma, a = Phi^-1(1 - k/n)), then bracketed
    Illinois-secant iterations on exact fp32 counts.  Counts use
    tensor_scalar(is_ge) with per-partition thresholds; cross-partition
    per-batch sums via a PE matmul against a half-selector matrix, which
    also broadcasts the result to all partitions.
  - partition layout: pieces of 64 row-blocks interleaved by batch, so in
    every [128, x] tile partitions 0..63 hold batch 0 and 64..127 batch 1.
  - final: per-partition accumulators -> ones-matmul column sums ->
    scalar loss per core; host sums the 8 partials.
"""
import sys

sys.path.insert(0, "/opt/trn_rl_repo")

import math
from statistics import NormalDist

import numpy as np

import concourse.bass as bass
import concourse.tile as tile
from concourse import mybir
from concourse import bass_utils
from concourse.bass_utils import run_bass_kernel_spmd

F32 = mybir.dt.float32
OP = mybir.AluOpType

# Artifact upload needs a bucket; keep traces local.
bass_utils.upload_artifacts = lambda tmpdir: f"local:{tmpdir}"


def _patched_drain_and_barrier(self, tick_clock, wait_clock):
    # This walrus build rejects >1 sync-wait on CTRL instructions ("Too many
    # sync wait commands"); split the tail-drain waits into single-wait NOPs.
    nc = self.nc
    drain_inst = nc.sync.drain()
    wait_clock.add_sem_waits(
        drain_inst.ins, tile.ScopedClock({None: tick_clock.global_clock})
    )
    si = drain_inst.ins.sync_info
    waits = list(si.on_wait) if si is not None else []
    if len(waits) > 1:
        si.on_wait = []
        id2handle = {h.num: h for h in self.sems.allocated().values()}
        for w in waits:
            nc.sync.wait_ge(id2handle[w.id], w.wait_value)
    nc.all_engine_barrier()
    popped = nc._tile_sem_poison_stack.pop()
    assert popped is self._sem_poison
    nc.clear_and_free_semaphores(list(self.sems.allocated().values()))
    nc.all_engine_barrier()


tile.TileContext._drain_and_barrier = _patched_drain_and_barrier

_NOP_CLS = None
_split_ctr = [0]


def _split_multi_waits(nc):
    """This walrus build allows at most one sync-wait per instruction; peel
    extra waits onto single-wait NOPs inserted just before, on the same
    engine."""
    global _NOP_CLS
    if _NOP_CLS is None:
        import bass_rust

        _NOP_CLS = bass_rust.InstNoOp
    import bass_rust

    for f in nc.m.functions:
        for blk in f.blocks:
            insts = blk.instructions
            out = []
            changed = False
            for ins in insts:
                si = ins.sync_info
                if si is not None and len(si.on_wait) > 1:
                    waits = list(si.on_wait)
                    for w in waits[:-1]:
                        _split_ctr[0] += 1
                        nop = _NOP_CLS(name=f"wsplit_{_split_ctr[0]}")
                        nop.engine = ins.engine
                        nop.sync_info = bass_rust.SyncInfo(
                            on_wait=[w], on_update=[]
                        )
                        out.append(nop)
                    si.on_wait = [waits[-1]]
                    changed = True
                out.append(ins)
            if changed:
                blk.instructions = out

# Problem geometry (hardcoded per spec nn_CHSLoss_75582834475514)
POOL = 8
B, H, W = 16, 192, 256  # full batch, pooled map height/width
N_CORES = 8
BPC = B // N_CORES      # batches per core = 2
NPB = H * W             # elements per batch row = 49152
PIECES = H // 64        # 3 pieces of 64 row-blocks per batch


def build_program(num, weight, a_const, delta, n_iter, w=W, debug=False,
                  split_waits=True):
    """Build the per-core Bass program.  `w` is the pooled width (reduced in
    sim tests); gt width is w*POOL."""
    gw = w * POOL
    npb = H * w
    cols = PIECES * w  # free size of full per-map tensors

    nc = bass.Bass("TRN2", target_bir_lowering=False, debug=False, num_devices=1)
    map0_t = nc.dram_tensor("map0", [BPC * H, w], F32, kind="ExternalInput")
    map1_t = nc.dram_tensor("map1", [BPC * H, w], F32, kind="ExternalInput")
    gt_t = nc.dram_tensor("gt", [BPC * H * POOL, gw], F32, kind="ExternalInput")
    consts_t = nc.dram_tensor("consts", [128, 225], F32, kind="ExternalInput")
    loss_t = nc.dram_tensor("loss", [1, 1], F32, kind="ExternalOutput")
    dbg_t = (
        nc.dram_tensor("dbg", [128, 12], F32, kind="ExternalOutput")
        if debug
        else None
    )

    with tile.TileContext(nc) as tc:
        with (
            tc.tile_pool(name="big", bufs=1) as big,
            tc.tile_pool(name="chk", bufs=6) as chp,
            tc.tile_pool(name="small", bufs=1) as small,
            tc.tile_pool(name="it", bufs=2) as itp,
            tc.tile_pool(name="p8", bufs=1, space="PSUM") as p8p,
            tc.tile_pool(name="psum", bufs=1, space="PSUM") as psp,
        ):
            # ---- constants (host-generated: partition-offset memsets are
            # not supported): cols 0:32 BD4, 32:96 PD, 96:224 halfsel,
            # 224:225 ones
            CONSTS = small.tile([128, 225], F32, tag="CONSTS")
            nc.sync.dma_start(CONSTS[:], consts_t.ap()[:])
            BD4 = CONSTS[:, 0:32]
            PD = CONSTS[:, 32:96]
            halfsel = CONSTS[:, 96:224]
            ones = CONSTS[:, 224:225]

            # ---- persistent per-element tensors [128, cols]
            m0 = big.tile([128, cols], F32, tag="m0")
            m1 = big.tile([128, cols], F32, tag="m1")
            Pg = big.tile([128, cols], F32, tag="Pg")
            d0 = big.tile([128, cols], F32, tag="d0")
            d1 = big.tile([128, cols], F32, tag="d1")
            err0 = big.tile([128, cols], F32, tag="err0")
            err1 = big.tile([128, cols], F32, tag="err1")
            dsq0 = big.tile([128, cols], F32, tag="dsq0")
            dsq1 = big.tile([128, cols], F32, tag="dsq1")
            diff0 = big.tile([128, cols], F32, tag="diff0")
            diff1 = big.tile([128, cols], F32, tag="diff1")
            scr = big.tile([128, cols], F32, tag="scr")

            # per-partition reduction accumulators:
            # SEQ cols: [sum(err0), sum(err1), sum(dsq0), sum(dsq1)]
            SEQ = small.tile([128, 4], F32, tag="SEQ")
            # MD cols: [sum(mask0*diff0), sum(mask1*diff1)]
            MD = small.tile([128, 2], F32, tag="MD")

            # map views: [2, 192, w] batch-major in DRAM
            m0v = map0_t.ap().rearrange("(b r) c -> b r c", b=BPC)
            m1v = map1_t.ap().rearrange("(b r) c -> b r c", b=BPC)
            map_dmas = []
            for x in range(PIECES):
                s = slice(x * w, (x + 1) * w)
                rsl = slice(64 * x, 64 * (x + 1))
                map_dmas.append((m0[:, s], m0v[:, rsl, :]))
                map_dmas.append((m1[:, s], m1v[:, rsl, :]))

            # ---- pooling: plain full-rate loads of [128, gw] row-chunks;
            # 4-row sums on PE (block-diagonal matmul, M=32, so PSUM write
            # bases stay 32-aligned) with 4 chunks stacked into one
            # [128, gw] PSUM tile; one 3D col-pool reduce per group; a tiny
            # PE pair-sum matmul completes the 8-row pool.
            # (Accumulate-DMA row-pooling is CCE-RMW-limited to ~166 GB/s,
            # more than 2x slower than plain loads.)
            gtr = gt_t.ap()  # [BPC*H*POOL, gw] rows
            n_chunks = BPC * H * POOL // 128  # 24
            n_groups = n_chunks // 3  # 8 groups of 3 chunks = 48 row-blocks
            Pc = [
                small.tile([48, w], F32, tag=f"Pc{_x}", name=f"Pc{_x}")
                for _x in range(n_groups)
            ]
            # PSUM tiles are half-width (2 banks) so bufs=2 fits and PE can
            # run ahead of the col-pool without HAM-resetting idle gaps.
            ghw = gw // 2
            for g in range(n_groups):
                P8h = [
                    p8p.tile([96, ghw], F32, tag="P8", name=f"P8_{g}_{_h}")
                    for _h in range(2)
                ]
                for j in range(3):
                    jc = 3 * g + j
                    ch = chp.tile([128, gw], F32, tag="ch")
                    nc.sync.dma_start(ch[:], gtr[128 * jc:128 * (jc + 1), :])
                    for hh in range(2):
                        for qo in range(0, ghw, 512):
                            qw = min(512, ghw - qo)
                            nc.tensor.matmul(
                                P8h[hh][32 * j:32 * (j + 1), qo:qo + qw],
                                BD4[:], ch[:, hh * ghw + qo:hh * ghw + qo + qw],
                                start=True, stop=True,
                            )
                Pc4 = itp.tile([96, w], F32, tag="Pc4")
                for hh in range(2):
                    nc.vector.reduce_sum(
                        Pc4[:, hh * (w // 2):(hh + 1) * (w // 2)],
                        P8h[hh][:].rearrange("p (g q) -> p g q", q=POOL),
                        axis=mybir.AxisListType.X,
                    )
                PS2 = psp.tile([48, w], F32, tag="PS2")
                nc.tensor.matmul(PS2[:], PD[0:96, 0:48], Pc4[:], start=True, stop=True)  # noqa: E501
                nc.scalar.copy(Pc[g][:], PS2[:])
            for dst, src in map_dmas:
                nc.sync.dma_start(dst, src)
            # group g holds rb' = 48g..48g+48 (contiguous, rb' = b*H + rb).
            # Piece x of the batch-interleaved layout needs rb' in
            # [b*H + 64x, b*H + 64x + 64) at partitions 64b..64b+64.
            for x in range(PIECES):
                s = slice(x * w, (x + 1) * w)
                for b in range(2):
                    lo_rbp = b * H + 64 * x
                    done = 0
                    while done < 64:
                        rbp = lo_rbp + done
                        g = rbp // 48
                        off = rbp % 48
                        take = min(48 - off, 64 - done)
                        nc.sync.dma_start(
                            Pg[64 * b + done:64 * b + done + take, s],
                            Pc[g][off:off + take, :],
                        )
                        done += take

            # ---- elementwise stages, per piece
            wneg = -float(weight)
            for x in range(PIECES):
                s = slice(x * w, (x + 1) * w)
                nc.vector.tensor_sub(d0[:, s], m0[:, s], Pg[:, s])
                nc.vector.tensor_sub(d1[:, s], m1[:, s], Pg[:, s])
                nc.vector.scalar_tensor_tensor(
                    err0[:, s], d0[:, s], -1.0, d0[:, s], op0=OP.mult, op1=OP.max
                )
                nc.vector.scalar_tensor_tensor(
                    err1[:, s], d1[:, s], -1.0, d1[:, s], op0=OP.mult, op1=OP.max
                )
                nc.scalar.square(dsq0[:, s], d0[:, s])
                nc.scalar.square(dsq1[:, s], d1[:, s])
                if num >= 1:
                    e0x = itp.tile([128, w], F32, tag="e0x")
                    e1x = itp.tile([128, w], F32, tag="e1x")
                    nc.vector.scalar_tensor_tensor(
                        e0x[:], d1[:, s], wneg, d0[:, s], op0=OP.mult, op1=OP.add
                    )
                    nc.vector.scalar_tensor_tensor(
                        e1x[:], d0[:, s], wneg, d1[:, s], op0=OP.mult, op1=OP.add
                    )
                    esq0x = itp.tile([128, w], F32, tag="esq0x")
                    esq1x = itp.tile([128, w], F32, tag="esq1x")
                    nc.scalar.square(esq0x[:], e0x[:])
                    nc.scalar.square(esq1x[:], e1x[:])
                    nc.vector.tensor_sub(diff0[:, s], esq0x[:], dsq0[:, s])
                    nc.vector.tensor_sub(diff1[:, s], esq1x[:], dsq1[:, s])

            # ---- per-partition sums for stats + loss base
            nc.vector.reduce_sum(SEQ[:, 0:1], err0[:], axis=mybir.AxisListType.X)
            nc.vector.reduce_sum(SEQ[:, 1:2], err1[:], axis=mybir.AxisListType.X)
            nc.vector.reduce_sum(SEQ[:, 2:3], dsq0[:], axis=mybir.AxisListType.X)
            nc.vector.reduce_sum(SEQ[:, 3:4], dsq1[:], axis=mybir.AxisListType.X)

            if num >= 1:
                # ---- stats -> initial bracket [t0 - delta, t0 + delta]
                Sst = psp.tile([128, 4], F32, tag="Sst")
                nc.tensor.matmul(Sst[:], halfsel[:], SEQ[:], start=True, stop=True)
                mu = small.tile([128, 2], F32, tag="mu")
                ex2 = small.tile([128, 2], F32, tag="ex2")
                inv_n = 1.0 / float(npb)
                nc.vector.tensor_scalar(mu[:], Sst[:, 0:2], inv_n, None, OP.mult)
                nc.vector.tensor_scalar(ex2[:], Sst[:, 2:4], inv_n, None, OP.mult)
                var = small.tile([128, 2], F32, tag="var")
                nc.vector.tensor_mul(var[:], mu[:], mu[:])
                nc.vector.tensor_sub(var[:], ex2[:], var[:])
                sig = small.tile([128, 2], F32, tag="sig")
                nc.scalar.sqrt(sig[:], var[:])
                t0 = small.tile([128, 2], F32, tag="t0")
                nc.vector.scalar_tensor_tensor(
                    t0[:], sig[:], float(a_const), mu[:], op0=OP.mult, op1=OP.add
                )
                lo = small.tile([128, 2], F32, tag="lo")
                hi = small.tile([128, 2], F32, tag="hi")
                tcur = small.tile([128, 2], F32, tag="tcur")
                flo = small.tile([128, 2], F32, tag="flo")
                fhi = small.tile([128, 2], F32, tag="fhi")
                nc.vector.tensor_scalar(lo[:], t0[:], float(delta), None, OP.subtract)
                nc.vector.tensor_scalar(hi[:], t0[:], float(delta), None, OP.add)
                nc.vector.tensor_copy(tcur[:], t0[:])
                nc.vector.memset(flo[:], float(npb - num))
                nc.vector.memset(fhi[:], float(-num))

                # ---- Illinois-secant iterations on exact counts
                for it in range(n_iter):
                    Cc = itp.tile([128, 2], F32, tag="Cc")
                    nc.vector.tensor_scalar(
                        scr[:], err0[:], tcur[:, 0:1], None, OP.is_ge, OP.add,
                        accum_out=Cc[:, 0:1],
                    )
                    nc.vector.tensor_scalar(
                        scr[:], err1[:], tcur[:, 1:2], None, OP.is_ge, OP.add,
                        accum_out=Cc[:, 1:2],
                    )
                    Scnt = psp.tile([128, 2], F32, tag="Scnt")
                    nc.tensor.matmul(Scnt[:], halfsel[:], Cc[:], start=True, stop=True)
                    ft = itp.tile([128, 2], F32, tag="ft")
                    ge = itp.tile([128, 2], mybir.dt.int8, tag="ge")
                    nge = itp.tile([128, 2], mybir.dt.int8, tag="nge")
                    nc.vector.tensor_scalar(ft[:], Scnt[:], float(num), None, OP.subtract)
                    nc.vector.tensor_scalar(ge[:], ft[:], 0.0, None, OP.is_ge)
                    nc.vector.tensor_scalar(nge[:], ft[:], 0.0, None, OP.is_lt)
                    # lo,flo <- t,ft when count>=k ; hi,fhi <- t,ft otherwise;
                    # the retained side's f halves (Illinois).
                    nc.vector.copy_predicated(lo[:], ge[:], tcur[:])
                    nc.vector.copy_predicated(hi[:], nge[:], tcur[:])
                    nc.vector.tensor_scalar(flo[:], flo[:], 0.5, None, OP.mult)
                    nc.vector.copy_predicated(flo[:], ge[:], ft[:])
                    nc.vector.tensor_scalar(fhi[:], fhi[:], 0.5, None, OP.mult)
                    nc.vector.copy_predicated(fhi[:], nge[:], ft[:])
                    if it + 1 < n_iter:
                        den = itp.tile([128, 2], F32, tag="den")
                        rec = itp.tile([128, 2], F32, tag="rec")
                        frac = itp.tile([128, 2], F32, tag="frac")
                        stp = itp.tile([128, 2], F32, tag="stp")
                        nc.vector.tensor_sub(den[:], flo[:], fhi[:])
                        nc.vector.reciprocal(rec[:], den[:])
                        nc.vector.tensor_mul(frac[:], flo[:], rec[:])
                        nc.vector.tensor_sub(stp[:], hi[:], lo[:])
                        nc.vector.tensor_mul(stp[:], frac[:], stp[:])
                        nc.vector.tensor_add(tcur[:], lo[:], stp[:])

                # ---- masked sums with final thresholds (= lo)
                nc.vector.scalar_tensor_tensor(
                    scr[:], err0[:], lo[:, 0:1], diff0[:],
                    op0=OP.is_ge, op1=OP.mult, accum_out=MD[:, 0:1],
                )
                nc.vector.scalar_tensor_tensor(
                    scr[:], err1[:], lo[:, 1:2], diff1[:],
                    op0=OP.is_ge, op1=OP.mult, accum_out=MD[:, 1:2],
                )

                if dbg_t is not None:
                    dbg = small.tile([128, 12], F32, tag="dbg")
                    nc.vector.tensor_copy(dbg[:, 0:2], mu[:])
                    nc.vector.tensor_copy(dbg[:, 2:4], sig[:])
                    nc.vector.tensor_copy(dbg[:, 4:6], t0[:])
                    nc.vector.tensor_copy(dbg[:, 6:8], lo[:])
                    nc.vector.tensor_copy(dbg[:, 8:10], SEQ[:, 0:2])
                    nc.vector.tensor_copy(dbg[:, 10:12], MD[:])
                    nc.sync.dma_start(dbg_t.ap()[:], dbg[:])

            # ---- final reduction: loss = sum over partitions of
            #      dsq0+dsq1 (base) + masked diffs
            Sfin = psp.tile([1, 4], F32, tag="Sfin")
            nc.tensor.matmul(Sfin[:, 0:2], ones[:], SEQ[:, 2:4], start=True, stop=True)
            if num >= 1:
                nc.tensor.matmul(Sfin[:, 2:4], ones[:], MD[:], start=True, stop=True)
            else:
                pass
            outT = small.tile([1, 1], F32, tag="outT")
            ncols = 4 if num >= 1 else 2
            nc.vector.reduce_sum(outT[:], Sfin[:, 0:ncols], axis=mybir.AxisListType.X)
            nc.sync.dma_start(loss_t.ap()[:], outT[:])

    if split_waits:
        # CoreSim's race detector rejects the raw NOPs, so sim builds skip
        # this; the HW compile path requires it.
        _split_multi_waits(nc)
    return nc


_build_cache = {}


def _get_program(num, weight, w=W):
    key = (num, float(weight), w)
    if key not in _build_cache:
        npb = H * w
        if num >= 1:
            q = 1.0 - num / float(npb)
            a_const = NormalDist().inv_cdf(q)
            delta = 0.75 if num >= 1000 else 1.5
            n_iter = 12 if num >= 1000 else 16
        else:
            a_const, delta, n_iter = 0.0, 0.0, 0
        _build_cache[key] = build_program(num, weight, a_const, delta, n_iter, w=w)
    return _build_cache[key]


def make_consts():
    c = np.zeros((128, 225), np.float32)
    for m in range(32):           # BD4: sum groups of 4 partitions
        c[4 * m:4 * m + 4, m] = 1.0
    for m in range(64):           # PD: sum partition pairs
        c[2 * m:2 * m + 2, 32 + m] = 1.0
    c[0:64, 96:160] = 1.0         # halfsel upper-left block
    c[64:128, 160:224] = 1.0      # halfsel lower-right block
    c[:, 224] = 1.0               # ones
    return c


def make_in_maps(map0, map1, gt_density, w=W):
    gw = w * POOL
    m0 = np.ascontiguousarray(np.asarray(map0, dtype=np.float32)).reshape(B, H, w)
    m1 = np.ascontiguousarray(np.asarray(map1, dtype=np.float32)).reshape(B, H, w)
    gt = np.ascontiguousarray(np.asarray(gt_density, dtype=np.float32)).reshape(
        B, H * POOL, gw
    )
    cst = make_consts()
    in_maps = []
    for c in range(N_CORES):
        bs = slice(c * BPC, (c + 1) * BPC)
        in_maps.append(
            {
                "map0": m0[bs].reshape(BPC * H, w),
                "map1": m1[bs].reshape(BPC * H, w),
                "gt": gt[bs].reshape(BPC * H * POOL, gw),
                "consts": cst,
            }
        )
    return in_maps


def kernel(map0, map1, gt_density, process):
    p = float(process)
    weight = 1.0 * p
    noisy_ratio = 0.1 * p
    num = int(H * W * noisy_ratio)
    nc = _get_program(num, weight)
    in_maps = make_in_maps(map0, map1, gt_density)
    res = run_bass_kernel_spmd(nc, in_maps, list(range(N_CORES)))
    total = 0.0
    for c in range(N_CORES):
        total += float(res.results[c]["loss"][0, 0])
    return np.float32(total)


# revision 30
# speedup vs baseline: 1.2258x; 1.2258x over previous
"""Trainium2 Bass kernel for CHSLoss (top-k masked MSE), 8-core data parallel.

Math (per batch row, n = H*W elements, k = int(n * 0.1 * process)):
    gt   = 8x8 sum-pool of gt_density
    d_i  = map_i - gt,  err_i = |d_i|
    mask_i = err_i >= (k-th largest of err_i)
    loss += sum(d_i^2) + sum(mask_i * ((d_i - w*d_j)^2 - d_i^2))   (j != i)

Device strategy per core (2 batches/core):
  - row-pool via SWDGE accumulate-DMA (8 passes, CCE add), col-pool via a
    3D free-dim reduce -> pooled gt in SBUF; all elementwise stages on
    DVE/ACT.
  - threshold search: per-(map,batch) Gaussian-stats initial bracket
    (t0 = mu + a*sigma, a = Phi^-1(1 - k/n)), then bracketed
    Illinois-secant iterations on exact fp32 counts.  Counts use
    tensor_scalar(is_ge) with per-partition thresholds; cross-partition
    per-batch sums via a PE matmul against a half-selector matrix, which
    also broadcasts the result to all partitions.
  - partition layout: pieces of 64 row-blocks interleaved by batch, so in
    every [128, x] tile partitions 0..63 hold batch 0 and 64..127 batch 1.
  - final: per-partition accumulators -> ones-matmul column sums ->
    scalar loss per core; host sums the 8 partials.
"""
import sys

sys.path.insert(0, "/opt/trn_rl_repo")

import math
from statistics import NormalDist

import numpy as np

import concourse.bass as bass
import concourse.tile as tile
from concourse import mybir
from concourse import bass_utils
from concourse.bass_utils import run_bass_kernel_spmd

F32 = mybir.dt.float32
OP = mybir.AluOpType

# Artifact upload needs a bucket; keep traces local.
bass_utils.upload_artifacts = lambda tmpdir: f"local:{tmpdir}"


def _patched_drain_and_barrier(self, tick_clock, wait_clock):
    # This walrus build rejects >1 sync-wait on CTRL instructions ("Too many
    # sync wait commands"); split the tail-drain waits into single-wait NOPs.
    nc = self.nc
    drain_inst = nc.sync.drain()
    wait_clock.add_sem_waits(
        drain_inst.ins, tile.ScopedClock({None: tick_clock.global_clock})
    )
    si = drain_inst.ins.sync_info
    waits = list(si.on_wait) if si is not None else []
    if len(waits) > 1:
        si.on_wait = []
        id2handle = {h.num: h for h in self.sems.allocated().values()}
        for w in waits:
            nc.sync.wait_ge(id2handle[w.id], w.wait_value)
    nc.all_engine_barrier()
    popped = nc._tile_sem_poison_stack.pop()
    assert popped is self._sem_poison
    nc.clear_and_free_semaphores(list(self.sems.allocated().values()))
    nc.all_engine_barrier()


tile.TileContext._drain_and_barrier = _patched_drain_and_barrier

_NOP_CLS = None
_split_ctr = [0]


def _split_multi_waits(nc):
    """This walrus build allows at most one sync-wait per instruction; peel
    extra waits onto single-wait NOPs inserted just before, on the same
    engine."""
    global _NOP_CLS
    if _NOP_CLS is None:
        import bass_rust

        _NOP_CLS = bass_rust.InstNoOp
    import bass_rust

    for f in nc.m.functions:
        for blk in f.blocks:
            insts = blk.instructions
            out = []
            changed = False
            for ins in insts:
                si = ins.sync_info
                if si is not None and len(si.on_wait) > 1:
                    waits = list(si.on_wait)
                    for w in waits[:-1]:
                        _split_ctr[0] += 1
                        nop = _NOP_CLS(name=f"wsplit_{_split_ctr[0]}")
                        nop.engine = ins.engine
                        nop.sync_info = bass_rust.SyncInfo(
                            on_wait=[w], on_update=[]
                        )
                        out.append(nop)
                    si.on_wait = [waits[-1]]
                    changed = True
                out.append(ins)
            if changed:
                blk.instructions = out

# Problem geometry (hardcoded per spec nn_CHSLoss_75582834475514)
POOL = 8
B, H, W = 16, 192, 256  # full batch, pooled map height/width
N_CORES = 8
BPC = B // N_CORES      # batches per core = 2
NPB = H * W             # elements per batch row = 49152
PIECES = H // 64        # 3 pieces of 64 row-blocks per batch


def build_program(num, weight, a_const, delta, n_iter, w=W, debug=False,
                  split_waits=True):
    """Build the per-core Bass program.  `w` is the pooled width (reduced in
    sim tests); gt width is w*POOL."""
    gw = w * POOL
    npb = H * w
    cols = PIECES * w  # free size of full per-map tensors

    nc = bass.Bass("TRN2", target_bir_lowering=False, debug=False, num_devices=1)
    map0_t = nc.dram_tensor("map0", [BPC * H, w], F32, kind="ExternalInput")
    map1_t = nc.dram_tensor("map1", [BPC * H, w], F32, kind="ExternalInput")
    gt_t = nc.dram_tensor("gt", [BPC * H * POOL, gw], F32, kind="ExternalInput")
    consts_t = nc.dram_tensor("consts", [128, 225], F32, kind="ExternalInput")
    loss_t = nc.dram_tensor("loss", [1, 1], F32, kind="ExternalOutput")
    dbg_t = (
        nc.dram_tensor("dbg", [128, 12], F32, kind="ExternalOutput")
        if debug
        else None
    )

    with tile.TileContext(nc) as tc:
        with (
            tc.tile_pool(name="big", bufs=1) as big,
            tc.tile_pool(name="chk", bufs=6) as chp,
            tc.tile_pool(name="small", bufs=1) as small,
            tc.tile_pool(name="it", bufs=2) as itp,
            tc.tile_pool(name="p8", bufs=2, space="PSUM") as p8p,
            tc.tile_pool(name="psum", bufs=1, space="PSUM") as psp,
        ):
            # ---- constants (host-generated: partition-offset memsets are
            # not supported): cols 0:32 BD4, 32:96 PD, 96:224 halfsel,
            # 224:225 ones
            CONSTS = small.tile([128, 225], F32, tag="CONSTS")
            nc.sync.dma_start(CONSTS[:], consts_t.ap()[:])
            BD4 = CONSTS[:, 0:32]
            PD = CONSTS[:, 32:96]
            halfsel = CONSTS[:, 96:224]
            ones = CONSTS[:, 224:225]

            # ---- persistent per-element tensors [128, cols]
            m0 = big.tile([128, cols], F32, tag="m0")
            m1 = big.tile([128, cols], F32, tag="m1")
            Pg = big.tile([128, cols], F32, tag="Pg")
            d0 = big.tile([128, cols], F32, tag="d0")
            d1 = big.tile([128, cols], F32, tag="d1")
            err0 = big.tile([128, cols], F32, tag="err0")
            err1 = big.tile([128, cols], F32, tag="err1")
            dsq0 = big.tile([128, cols], F32, tag="dsq0")
            dsq1 = big.tile([128, cols], F32, tag="dsq1")
            diff0 = big.tile([128, cols], F32, tag="diff0")
            diff1 = big.tile([128, cols], F32, tag="diff1")
            scr = big.tile([128, cols], F32, tag="scr")

            # per-partition reduction accumulators:
            # SEQ cols: [sum(err0), sum(err1), sum(dsq0), sum(dsq1)]
            SEQ = small.tile([128, 4], F32, tag="SEQ")
            # MD cols: [sum(mask0*diff0), sum(mask1*diff1)]
            MD = small.tile([128, 2], F32, tag="MD")

            # map views: [2, 192, w] batch-major in DRAM
            m0v = map0_t.ap().rearrange("(b r) c -> b r c", b=BPC)
            m1v = map1_t.ap().rearrange("(b r) c -> b r c", b=BPC)
            map_dmas = []
            for x in range(PIECES):
                s = slice(x * w, (x + 1) * w)
                rsl = slice(64 * x, 64 * (x + 1))
                map_dmas.append((m0[:, s], m0v[:, rsl, :]))
                map_dmas.append((m1[:, s], m1v[:, rsl, :]))

            # ---- pooling: plain full-rate loads of [128, gw] row-chunks;
            # 4-row sums on PE (block-diagonal matmul, M=32, so PSUM write
            # bases stay 32-aligned) with 4 chunks stacked into one
            # [128, gw] PSUM tile; one 3D col-pool reduce per group; a tiny
            # PE pair-sum matmul completes the 8-row pool.
            # (Accumulate-DMA row-pooling is CCE-RMW-limited to ~166 GB/s,
            # more than 2x slower than plain loads.)
            gtr = gt_t.ap()  # [BPC*H*POOL, gw] rows
            n_chunks = BPC * H * POOL // 128  # 24
            n_groups = n_chunks // 3  # 8 groups of 3 chunks = 48 row-blocks
            Pc = [
                small.tile([48, w], F32, tag=f"Pc{_x}", name=f"Pc{_x}")
                for _x in range(n_groups)
            ]
            # PSUM tiles are half-width (2 banks) so bufs=2 fits and PE can
            # run ahead of the col-pool without HAM-resetting idle gaps.
            ghw = gw // 2
            for g in range(n_groups):
                P8h = [
                    p8p.tile([96, ghw], F32, tag="P8", name=f"P8_{g}_{_h}")
                    for _h in range(2)
                ]
                for j in range(3):
                    jc = 3 * g + j
                    ch = chp.tile([128, gw], F32, tag="ch")
                    nc.sync.dma_start(ch[:], gtr[128 * jc:128 * (jc + 1), :])
                    for hh in range(2):
                        for qo in range(0, ghw, 512):
                            qw = min(512, ghw - qo)
                            nc.tensor.matmul(
                                P8h[hh][32 * j:32 * (j + 1), qo:qo + qw],
                                BD4[:], ch[:, hh * ghw + qo:hh * ghw + qo + qw],
                                start=True, stop=True,
                            )
                Pc4 = itp.tile([96, w], F32, tag="Pc4")
                for hh in range(2):
                    nc.vector.reduce_sum(
                        Pc4[:, hh * (w // 2):(hh + 1) * (w // 2)],
                        P8h[hh][:].rearrange("p (g q) -> p g q", q=POOL),
                        axis=mybir.AxisListType.X,
                    )
                PS2 = psp.tile([48, w], F32, tag="PS2")
                nc.tensor.matmul(PS2[:], PD[0:96, 0:48], Pc4[:], start=True, stop=True)  # noqa: E501
                nc.scalar.copy(Pc[g][:], PS2[:])
            for dst, src in map_dmas:
                nc.sync.dma_start(dst, src)
            # group g holds rb' = 48g..48g+48 (contiguous, rb' = b*H + rb).
            # Piece x of the batch-interleaved layout needs rb' in
            # [b*H + 64x, b*H + 64x + 64) at partitions 64b..64b+64.
            for x in range(PIECES):
                s = slice(x * w, (x + 1) * w)
                for b in range(2):
                    lo_rbp = b * H + 64 * x
                    done = 0
                    while done < 64:
                        rbp = lo_rbp + done
                        g = rbp // 48
                        off = rbp % 48
                        take = min(48 - off, 64 - done)
                        nc.sync.dma_start(
                            Pg[64 * b + done:64 * b + done + take, s],
                            Pc[g][off:off + take, :],
                        )
                        done += take

            # ---- elementwise stages, per piece
            wneg = -float(weight)
            for x in range(PIECES):
                s = slice(x * w, (x + 1) * w)
                nc.vector.tensor_sub(d0[:, s], m0[:, s], Pg[:, s])
                nc.vector.tensor_sub(d1[:, s], m1[:, s], Pg[:, s])
                nc.vector.scalar_tensor_tensor(
                    err0[:, s], d0[:, s], -1.0, d0[:, s], op0=OP.mult, op1=OP.max
                )
                nc.vector.scalar_tensor_tensor(
                    err1[:, s], d1[:, s], -1.0, d1[:, s], op0=OP.mult, op1=OP.max
                )
                nc.scalar.square(dsq0[:, s], d0[:, s])
                nc.scalar.square(dsq1[:, s], d1[:, s])
                if num >= 1:
                    e0x = itp.tile([128, w], F32, tag="e0x")
                    e1x = itp.tile([128, w], F32, tag="e1x")
                    nc.vector.scalar_tensor_tensor(
                        e0x[:], d1[:, s], wneg, d0[:, s], op0=OP.mult, op1=OP.add
                    )
                    nc.vector.scalar_tensor_tensor(
                        e1x[:], d0[:, s], wneg, d1[:, s], op0=OP.mult, op1=OP.add
                    )
                    esq0x = itp.tile([128, w], F32, tag="esq0x")
                    esq1x = itp.tile([128, w], F32, tag="esq1x")
                    nc.scalar.square(esq0x[:], e0x[:])
                    nc.scalar.square(esq1x[:], e1x[:])
                    nc.vector.tensor_sub(diff0[:, s], esq0x[:], dsq0[:, s])
                    nc.vector.tensor_sub(diff1[:, s], esq1x[:], dsq1[:, s])

            # ---- per-partition sums for stats + loss base
            nc.vector.reduce_sum(SEQ[:, 0:1], err0[:], axis=mybir.AxisListType.X)
            nc.vector.reduce_sum(SEQ[:, 1:2], err1[:], axis=mybir.AxisListType.X)
            nc.vector.reduce_sum(SEQ[:, 2:3], dsq0[:], axis=mybir.AxisListType.X)
            nc.vector.reduce_sum(SEQ[:, 3:4], dsq1[:], axis=mybir.AxisListType.X)

            if num >= 1:
                # ---- stats -> initial bracket [t0 - delta, t0 + delta]
                Sst = psp.tile([128, 4], F32, tag="Sst")
                nc.tensor.matmul(Sst[:], halfsel[:], SEQ[:], start=True, stop=True)
                mu = small.tile([128, 2], F32, tag="mu")
                ex2 = small.tile([128, 2], F32, tag="ex2")
                inv_n = 1.0 / float(npb)
                nc.vector.tensor_scalar(mu[:], Sst[:, 0:2], inv_n, None, OP.mult)
                nc.vector.tensor_scalar(ex2[:], Sst[:, 2:4], inv_n, None, OP.mult)
                var = small.tile([128, 2], F32, tag="var")
                nc.vector.tensor_mul(var[:], mu[:], mu[:])
                nc.vector.tensor_sub(var[:], ex2[:], var[:])
                sig = small.tile([128, 2], F32, tag="sig")
                nc.scalar.sqrt(sig[:], var[:])
                t0 = small.tile([128, 2], F32, tag="t0")
                nc.vector.scalar_tensor_tensor(
                    t0[:], sig[:], float(a_const), mu[:], op0=OP.mult, op1=OP.add
                )
                lo = small.tile([128, 2], F32, tag="lo")
                hi = small.tile([128, 2], F32, tag="hi")
                tcur = small.tile([128, 2], F32, tag="tcur")
                flo = small.tile([128, 2], F32, tag="flo")
                fhi = small.tile([128, 2], F32, tag="fhi")
                nc.vector.tensor_scalar(lo[:], t0[:], float(delta), None, OP.subtract)
                nc.vector.tensor_scalar(hi[:], t0[:], float(delta), None, OP.add)
                nc.vector.tensor_copy(tcur[:], t0[:])
                nc.vector.memset(flo[:], float(npb - num))
                nc.vector.memset(fhi[:], float(-num))

                # ---- Illinois-secant iterations on exact counts
                for it in range(n_iter):
                    Cc = itp.tile([128, 2], F32, tag="Cc")
                    nc.vector.tensor_scalar(
                        scr[:], err0[:], tcur[:, 0:1], None, OP.is_ge, OP.add,
                        accum_out=Cc[:, 0:1],
                    )
                    nc.vector.tensor_scalar(
                        scr[:], err1[:], tcur[:, 1:2], None, OP.is_ge, OP.add,
                        accum_out=Cc[:, 1:2],
                    )
                    Scnt = psp.tile([128, 2], F32, tag="Scnt")
                    nc.tensor.matmul(Scnt[:], halfsel[:], Cc[:], start=True, stop=True)
                    ft = itp.tile([128, 2], F32, tag="ft")
                    ge = itp.tile([128, 2], mybir.dt.int8, tag="ge")
                    nge = itp.tile([128, 2], mybir.dt.int8, tag="nge")
                    nc.vector.tensor_scalar(ft[:], Scnt[:], float(num), None, OP.subtract)
                    nc.vector.tensor_scalar(ge[:], ft[:], 0.0, None, OP.is_ge)
                    nc.vector.tensor_scalar(nge[:], ft[:], 0.0, None, OP.is_lt)
                    # lo,flo <- t,ft when count>=k ; hi,fhi <- t,ft otherwise;
                    # the retained side's f halves (Illinois).
                    nc.vector.copy_predicated(lo[:], ge[:], tcur[:])
                    nc.vector.copy_predicated(hi[:], nge[:], tcur[:])
                    nc.vector.tensor_scalar(flo[:], flo[:], 0.5, None, OP.mult)
                    nc.vector.copy_predicated(flo[:], ge[:], ft[:])
                    nc.vector.tensor_scalar(fhi[:], fhi[:], 0.5, None, OP.mult)
                    nc.vector.copy_predicated(fhi[:], nge[:], ft[:])
                    if it + 1 < n_iter:
                        den = itp.tile([128, 2], F32, tag="den")
                        rec = itp.tile([128, 2], F32, tag="rec")
                        frac = itp.tile([128, 2], F32, tag="frac")
                        stp = itp.tile([128, 2], F32, tag="stp")
                        nc.vector.tensor_sub(den[:], flo[:], fhi[:])
                        nc.vector.reciprocal(rec[:], den[:])
                        nc.vector.tensor_mul(frac[:], flo[:], rec[:])
                        nc.vector.tensor_sub(stp[:], hi[:], lo[:])
                        nc.vector.tensor_mul(stp[:], frac[:], stp[:])
                        nc.vector.tensor_add(tcur[:], lo[:], stp[:])

                # ---- masked sums with final thresholds (= lo)
                nc.vector.scalar_tensor_tensor(
                    scr[:], err0[:], lo[:, 0:1], diff0[:],
                    op0=OP.is_ge, op1=OP.mult, accum_out=MD[:, 0:1],
                )
                nc.vector.scalar_tensor_tensor(
                    scr[:], err1[:], lo[:, 1:2], diff1[:],
                    op0=OP.is_ge, op1=OP.mult, accum_out=MD[:, 1:2],
                )

                if dbg_t is not None:
                    dbg = small.tile([128, 12], F32, tag="dbg")
                    nc.vector.tensor_copy(dbg[:, 0:2], mu[:])
                    nc.vector.tensor_copy(dbg[:, 2:4], sig[:])
                    nc.vector.tensor_copy(dbg[:, 4:6], t0[:])
                    nc.vector.tensor_copy(dbg[:, 6:8], lo[:])
                    nc.vector.tensor_copy(dbg[:, 8:10], SEQ[:, 0:2])
                    nc.vector.tensor_copy(dbg[:, 10:12], MD[:])
                    nc.sync.dma_start(dbg_t.ap()[:], dbg[:])

            # ---- final reduction: loss = sum over partitions of
            #      dsq0+dsq1 (base) + masked diffs
            Sfin = psp.tile([1, 4], F32, tag="Sfin")
            nc.tensor.matmul(Sfin[:, 0:2], ones[:], SEQ[:, 2:4], start=True, stop=True)
            if num >= 1:
                nc.tensor.matmul(Sfin[:, 2:4], ones[:], MD[:], start=True, stop=True)
            else:
                pass
            outT = small.tile([1, 1], F32, tag="outT")
            ncols = 4 if num >= 1 else 2
            nc.vector.reduce_sum(outT[:], Sfin[:, 0:ncols], axis=mybir.AxisListType.X)
            nc.sync.dma_start(loss_t.ap()[:], outT[:])

    if split_waits:
        # CoreSim's race detector rejects the raw NOPs, so sim builds skip
        # this; the HW compile path requires it.
        _split_multi_waits(nc)
    return nc


_build_cache = {}


def _get_program(num, weight, w=W):
    key = (num, float(weight), w)
    if key not in _build_cache:
        npb = H * w
        if num >= 1:
            q = 1.0 - num / float(npb)
            a_const = NormalDist().inv_cdf(q)
            delta = 0.75 if num >= 1000 else 1.5
            n_iter = 12 if num >= 1000 else 16
        else:
            a_const, delta, n_iter = 0.0, 0.0, 0
        _build_cache[key] = build_program(num, weight, a_const, delta, n_iter, w=w)
    return _build_cache[key]


def make_consts():
    c = np.zeros((128, 225), np.float32)
    for m in range(32):           # BD4: sum groups of 4 partitions
        c[4 * m:4 * m + 4, m] = 1.0
    for m in range(64):           # PD: sum partition pairs
        c[2 * m:2 * m + 2, 32 + m] = 1.0
    c[0:64, 96:160] = 1.0         # halfsel upper-left block
    c[64:128, 160:224] = 1.0      # halfsel lower-right block
    c[:, 224] = 1.0               # ones
    return c


def make_in_maps(map0, map1, gt_density, w=W):
    gw = w * POOL
    m0 = np.ascontiguousarray(np.asarray(map0, dtype=np.float32)).reshape(B, H, w)
    m1 = np.ascontiguousarray(np.asarray(map1, dtype=np.float32)).reshape(B, H, w)
    gt = np.ascontiguousarray(np.asarray(gt_density, dtype=np.float32)).reshape(
        B, H * POOL, gw
    )
    cst = make_consts()
    in_maps = []
    for c in range(N_CORES):
        bs = slice(c * BPC, (c + 1) * BPC)
        in_maps.append(
            {
                "map0": m0[bs].reshape(BPC * H, w),
                "map1": m1[bs].reshape(BPC * H, w),
                "gt": gt[bs].reshape(BPC * H * POOL, gw),
                "consts": cst,
            }
        )
    return in_maps


def kernel(map0, map1, gt_density, process):
    p = float(process)
    weight = 1.0 * p
    noisy_ratio = 0.1 * p
    num = int(H * W * noisy_ratio)
    nc = _get_program(num, weight)
    in_maps = make_in_maps(map0, map1, gt_density)
    res = run_bass_kernel_spmd(nc, in_maps, list(range(N_CORES)))
    total = 0.0
    for c in range(N_CORES):
        total += float(res.results[c]["loss"][0, 0])
    return np.float32(total)


# revision 32
# speedup vs baseline: 1.2868x; 1.0497x over previous
"""Trainium2 Bass kernel for CHSLoss (top-k masked MSE), 8-core data parallel.

Math (per batch row, n = H*W elements, k = int(n * 0.1 * process)):
    gt   = 8x8 sum-pool of gt_density
    d_i  = map_i - gt,  err_i = |d_i|
    mask_i = err_i >= (k-th largest of err_i)
    loss += sum(d_i^2) + sum(mask_i * ((d_i - w*d_j)^2 - d_i^2))   (j != i)

Device strategy per core (2 batches/core):
  - row-pool via SWDGE accumulate-DMA (8 passes, CCE add), col-pool via a
    3D free-dim reduce -> pooled gt in SBUF; all elementwise stages on
    DVE/ACT.
  - threshold search: per-(map,batch) Gaussian-stats initial bracket
    (t0 = mu + a*sigma, a = Phi^-1(1 - k/n)), then bracketed
    Illinois-secant iterations on exact fp32 counts.  Counts use
    tensor_scalar(is_ge) with per-partition thresholds; cross-partition
    per-batch sums via a PE matmul against a half-selector matrix, which
    also broadcasts the result to all partitions.
  - partition layout: pieces of 64 row-blocks interleaved by batch, so in
    every [128, x] tile partitions 0..63 hold batch 0 and 64..127 batch 1.
  - final: per-partition accumulators -> ones-matmul column sums ->
    scalar loss per core; host sums the 8 partials.
"""
import sys

sys.path.insert(0, "/opt/trn_rl_repo")

import math
from statistics import NormalDist

import numpy as np

import concourse.bass as bass
import concourse.tile as tile
from concourse import mybir
from concourse import bass_utils
from concourse.bass_utils import run_bass_kernel_spmd

F32 = mybir.dt.float32
OP = mybir.AluOpType

# Artifact upload needs a bucket; keep traces local.
bass_utils.upload_artifacts = lambda tmpdir: f"local:{tmpdir}"


def _patched_drain_and_barrier(self, tick_clock, wait_clock):
    # This walrus build rejects >1 sync-wait on CTRL instructions ("Too many
    # sync wait commands"); split the tail-drain waits into single-wait NOPs.
    nc = self.nc
    drain_inst = nc.sync.drain()
    wait_clock.add_sem_waits(
        drain_inst.ins, tile.ScopedClock({None: tick_clock.global_clock})
    )
    si = drain_inst.ins.sync_info
    waits = list(si.on_wait) if si is not None else []
    if len(waits) > 1:
        si.on_wait = []
        id2handle = {h.num: h for h in self.sems.allocated().values()}
        for w in waits:
            nc.sync.wait_ge(id2handle[w.id], w.wait_value)
    nc.all_engine_barrier()
    popped = nc._tile_sem_poison_stack.pop()
    assert popped is self._sem_poison
    nc.clear_and_free_semaphores(list(self.sems.allocated().values()))
    nc.all_engine_barrier()


tile.TileContext._drain_and_barrier = _patched_drain_and_barrier

_NOP_CLS = None
_split_ctr = [0]


def _split_multi_waits(nc):
    """This walrus build allows at most one sync-wait per instruction; peel
    extra waits onto single-wait NOPs inserted just before, on the same
    engine."""
    global _NOP_CLS
    if _NOP_CLS is None:
        import bass_rust

        _NOP_CLS = bass_rust.InstNoOp
    import bass_rust

    for f in nc.m.functions:
        for blk in f.blocks:
            insts = blk.instructions
            out = []
            changed = False
            for ins in insts:
                si = ins.sync_info
                if si is not None and len(si.on_wait) > 1:
                    waits = list(si.on_wait)
                    for w in waits[:-1]:
                        _split_ctr[0] += 1
                        nop = _NOP_CLS(name=f"wsplit_{_split_ctr[0]}")
                        nop.engine = ins.engine
                        nop.sync_info = bass_rust.SyncInfo(
                            on_wait=[w], on_update=[]
                        )
                        out.append(nop)
                    si.on_wait = [waits[-1]]
                    changed = True
                out.append(ins)
            if changed:
                blk.instructions = out

# Problem geometry (hardcoded per spec nn_CHSLoss_75582834475514)
POOL = 8
B, H, W = 16, 192, 256  # full batch, pooled map height/width
N_CORES = 8
BPC = B // N_CORES      # batches per core = 2
NPB = H * W             # elements per batch row = 49152
PIECES = H // 64        # 3 pieces of 64 row-blocks per batch


def build_program(num, weight, a_const, delta, n_iter, w=W, debug=False,
                  split_waits=True):
    """Build the per-core Bass program.  `w` is the pooled width (reduced in
    sim tests); gt width is w*POOL."""
    gw = w * POOL
    npb = H * w
    cols = PIECES * w  # free size of full per-map tensors

    nc = bass.Bass("TRN2", target_bir_lowering=False, debug=False, num_devices=1)
    map0_t = nc.dram_tensor("map0", [BPC * H, w], F32, kind="ExternalInput")
    map1_t = nc.dram_tensor("map1", [BPC * H, w], F32, kind="ExternalInput")
    gt_t = nc.dram_tensor("gt", [BPC * H * POOL, gw], F32, kind="ExternalInput")
    consts_t = nc.dram_tensor("consts", [128, 225], F32, kind="ExternalInput")
    loss_t = nc.dram_tensor("loss", [1, 1], F32, kind="ExternalOutput")
    dbg_t = (
        nc.dram_tensor("dbg", [128, 12], F32, kind="ExternalOutput")
        if debug
        else None
    )

    with tile.TileContext(nc) as tc:
        with (
            tc.tile_pool(name="big", bufs=1) as big,
            tc.tile_pool(name="chk", bufs=6) as chp,
            tc.tile_pool(name="small", bufs=1) as small,
            tc.tile_pool(name="it", bufs=2) as itp,
            tc.tile_pool(name="p8", bufs=2, space="PSUM") as p8p,
            tc.tile_pool(name="psum", bufs=1, space="PSUM") as psp,
        ):
            # ---- constants (host-generated: partition-offset memsets are
            # not supported): cols 0:32 BD4, 32:96 PD, 96:224 halfsel,
            # 224:225 ones
            CONSTS = small.tile([128, 225], F32, tag="CONSTS")
            nc.sync.dma_start(CONSTS[:], consts_t.ap()[:])
            BD4 = CONSTS[:, 0:32]
            PD = CONSTS[:, 32:96]
            halfsel = CONSTS[:, 96:224]
            ones = CONSTS[:, 224:225]

            # ---- persistent per-element tensors [128, cols]
            m0 = big.tile([128, cols], F32, tag="m0")
            m1 = big.tile([128, cols], F32, tag="m1")
            Pg = big.tile([128, cols], F32, tag="Pg")
            d0 = big.tile([128, cols], F32, tag="d0")
            d1 = big.tile([128, cols], F32, tag="d1")
            err0 = big.tile([128, cols], F32, tag="err0")
            err1 = big.tile([128, cols], F32, tag="err1")
            dsq0 = big.tile([128, cols], F32, tag="dsq0")
            dsq1 = big.tile([128, cols], F32, tag="dsq1")
            diff0 = big.tile([128, cols], F32, tag="diff0")
            diff1 = big.tile([128, cols], F32, tag="diff1")
            scr = big.tile([128, cols], F32, tag="scr")

            # per-partition reduction accumulators:
            # SEQ cols: [sum(err0), sum(err1), sum(dsq0), sum(dsq1)]
            SEQ = small.tile([128, 4], F32, tag="SEQ")
            # MD cols: [sum(mask0*diff0), sum(mask1*diff1)]
            MD = small.tile([128, 2], F32, tag="MD")

            # map views: [2, 192, w] batch-major in DRAM
            m0v = map0_t.ap().rearrange("(b r) c -> b r c", b=BPC)
            m1v = map1_t.ap().rearrange("(b r) c -> b r c", b=BPC)
            map_dmas = []
            for x in range(PIECES):
                s = slice(x * w, (x + 1) * w)
                rsl = slice(64 * x, 64 * (x + 1))
                map_dmas.append((m0[:, s], m0v[:, rsl, :]))
                map_dmas.append((m1[:, s], m1v[:, rsl, :]))

            # ---- pooling: plain full-rate loads of [128, gw] row-chunks;
            # 4-row sums on PE (block-diagonal matmul, M=32, so PSUM write
            # bases stay 32-aligned) with 4 chunks stacked into one
            # [128, gw] PSUM tile; one 3D col-pool reduce per group; a tiny
            # PE pair-sum matmul completes the 8-row pool.
            # (Accumulate-DMA row-pooling is CCE-RMW-limited to ~166 GB/s,
            # more than 2x slower than plain loads.)
            gtr = gt_t.ap()  # [BPC*H*POOL, gw] rows
            n_chunks = BPC * H * POOL // 128  # 24
            n_groups = n_chunks // 3  # 8 groups of 3 chunks = 48 row-blocks
            Pc = [
                small.tile([48, w], F32, tag=f"Pc{_x}", name=f"Pc{_x}")
                for _x in range(n_groups)
            ]
            # Alternate groups between two pooling paths to split work
            # between PE and DVE (fp32 matmuls cost 2 passes, so PE alone
            # cannot hide the full row-pool under the DMA):
            #  - PE-first: 4-row-sum matmuls on raw chunks (PE-heavy),
            #    then col-pool the [96, gw] PSUM on DVE.
            #  - DVE-first: col-pool each raw chunk on DVE (DVE-heavy),
            #    then cheap [*, w] 4-row-sum matmuls.
            ghw = gw // 2
            for g in range(n_groups):
                pe_first = (g % 2) == 0
                if pe_first:
                    P8h = [
                        p8p.tile([96, ghw], F32, tag="P8", name=f"P8_{g}_{_h}")
                        for _h in range(2)
                    ]
                    for j in range(3):
                        jc = 3 * g + j
                        ch = chp.tile([128, gw], F32, tag="ch")
                        nc.sync.dma_start(ch[:], gtr[128 * jc:128 * (jc + 1), :])
                        for hh in range(2):
                            for qo in range(0, ghw, 512):
                                qw = min(512, ghw - qo)
                                nc.tensor.matmul(
                                    P8h[hh][32 * j:32 * (j + 1), qo:qo + qw],
                                    BD4[:], ch[:, hh * ghw + qo:hh * ghw + qo + qw],
                                    start=True, stop=True,
                                )
                    Pc4 = itp.tile([96, w], F32, tag="Pc4")
                    for hh in range(2):
                        nc.vector.reduce_sum(
                            Pc4[:, hh * (w // 2):(hh + 1) * (w // 2)],
                            P8h[hh][:].rearrange("p (g q) -> p g q", q=POOL),
                            axis=mybir.AxisListType.X,
                        )
                else:
                    Q8 = psp.tile([96, w], F32, tag="Q8")
                    for j in range(3):
                        jc = 3 * g + j
                        ch = chp.tile([128, gw], F32, tag="ch")
                        nc.sync.dma_start(ch[:], gtr[128 * jc:128 * (jc + 1), :])
                        Cp = itp.tile([128, w], F32, tag="Cp")
                        nc.vector.reduce_sum(
                            Cp[:],
                            ch[:].rearrange("p (g q) -> p g q", q=POOL),
                            axis=mybir.AxisListType.X,
                        )
                        nc.tensor.matmul(
                            Q8[32 * j:32 * (j + 1), :], BD4[:], Cp[:],
                            start=True, stop=True,
                        )
                    Pc4 = itp.tile([96, w], F32, tag="Pc4")
                    nc.scalar.copy(Pc4[:], Q8[:])
                PS2 = psp.tile([48, w], F32, tag="PS2")
                nc.tensor.matmul(PS2[:], PD[0:96, 0:48], Pc4[:], start=True, stop=True)  # noqa: E501
                nc.scalar.copy(Pc[g][:], PS2[:])
            for dst, src in map_dmas:
                nc.sync.dma_start(dst, src)
            # group g holds rb' = 48g..48g+48 (contiguous, rb' = b*H + rb).
            # Piece x of the batch-interleaved layout needs rb' in
            # [b*H + 64x, b*H + 64x + 64) at partitions 64b..64b+64.
            for x in range(PIECES):
                s = slice(x * w, (x + 1) * w)
                for b in range(2):
                    lo_rbp = b * H + 64 * x
                    done = 0
                    while done < 64:
                        rbp = lo_rbp + done
                        g = rbp // 48
                        off = rbp % 48
                        take = min(48 - off, 64 - done)
                        nc.sync.dma_start(
                            Pg[64 * b + done:64 * b + done + take, s],
                            Pc[g][off:off + take, :],
                        )
                        done += take

            # ---- elementwise stages, per piece
            wneg = -float(weight)
            for x in range(PIECES):
                s = slice(x * w, (x + 1) * w)
                nc.vector.tensor_sub(d0[:, s], m0[:, s], Pg[:, s])
                nc.vector.tensor_sub(d1[:, s], m1[:, s], Pg[:, s])
                nc.vector.scalar_tensor_tensor(
                    err0[:, s], d0[:, s], -1.0, d0[:, s], op0=OP.mult, op1=OP.max
                )
                nc.vector.scalar_tensor_tensor(
                    err1[:, s], d1[:, s], -1.0, d1[:, s], op0=OP.mult, op1=OP.max
                )
                nc.scalar.square(dsq0[:, s], d0[:, s])
                nc.scalar.square(dsq1[:, s], d1[:, s])
                if num >= 1:
                    e0x = itp.tile([128, w], F32, tag="e0x")
                    e1x = itp.tile([128, w], F32, tag="e1x")
                    nc.vector.scalar_tensor_tensor(
                        e0x[:], d1[:, s], wneg, d0[:, s], op0=OP.mult, op1=OP.add
                    )
                    nc.vector.scalar_tensor_tensor(
                        e1x[:], d0[:, s], wneg, d1[:, s], op0=OP.mult, op1=OP.add
                    )
                    esq0x = itp.tile([128, w], F32, tag="esq0x")
                    esq1x = itp.tile([128, w], F32, tag="esq1x")
                    nc.scalar.square(esq0x[:], e0x[:])
                    nc.scalar.square(esq1x[:], e1x[:])
                    nc.vector.tensor_sub(diff0[:, s], esq0x[:], dsq0[:, s])
                    nc.vector.tensor_sub(diff1[:, s], esq1x[:], dsq1[:, s])

            # ---- per-partition sums for stats + loss base
            nc.vector.reduce_sum(SEQ[:, 0:1], err0[:], axis=mybir.AxisListType.X)
            nc.vector.reduce_sum(SEQ[:, 1:2], err1[:], axis=mybir.AxisListType.X)
            nc.vector.reduce_sum(SEQ[:, 2:3], dsq0[:], axis=mybir.AxisListType.X)
            nc.vector.reduce_sum(SEQ[:, 3:4], dsq1[:], axis=mybir.AxisListType.X)

            if num >= 1:
                # ---- stats -> initial bracket [t0 - delta, t0 + delta]
                Sst = psp.tile([128, 4], F32, tag="Sst")
                nc.tensor.matmul(Sst[:], halfsel[:], SEQ[:], start=True, stop=True)
                mu = small.tile([128, 2], F32, tag="mu")
                ex2 = small.tile([128, 2], F32, tag="ex2")
                inv_n = 1.0 / float(npb)
                nc.vector.tensor_scalar(mu[:], Sst[:, 0:2], inv_n, None, OP.mult)
                nc.vector.tensor_scalar(ex2[:], Sst[:, 2:4], inv_n, None, OP.mult)
                var = small.tile([128, 2], F32, tag="var")
                nc.vector.tensor_mul(var[:], mu[:], mu[:])
                nc.vector.tensor_sub(var[:], ex2[:], var[:])
                sig = small.tile([128, 2], F32, tag="sig")
                nc.scalar.sqrt(sig[:], var[:])
                t0 = small.tile([128, 2], F32, tag="t0")
                nc.vector.scalar_tensor_tensor(
                    t0[:], sig[:], float(a_const), mu[:], op0=OP.mult, op1=OP.add
                )
                lo = small.tile([128, 2], F32, tag="lo")
                hi = small.tile([128, 2], F32, tag="hi")
                tcur = small.tile([128, 2], F32, tag="tcur")
                flo = small.tile([128, 2], F32, tag="flo")
                fhi = small.tile([128, 2], F32, tag="fhi")
                nc.vector.tensor_scalar(lo[:], t0[:], float(delta), None, OP.subtract)
                nc.vector.tensor_scalar(hi[:], t0[:], float(delta), None, OP.add)
                nc.vector.tensor_copy(tcur[:], t0[:])
                nc.vector.memset(flo[:], float(npb - num))
                nc.vector.memset(fhi[:], float(-num))

                # ---- Illinois-secant iterations on exact counts
                for it in range(n_iter):
                    Cc = itp.tile([128, 2], F32, tag="Cc")
                    nc.vector.tensor_scalar(
                        scr[:], err0[:], tcur[:, 0:1], None, OP.is_ge, OP.add,
                        accum_out=Cc[:, 0:1],
                    )
                    nc.vector.tensor_scalar(
                        scr[:], err1[:], tcur[:, 1:2], None, OP.is_ge, OP.add,
                        accum_out=Cc[:, 1:2],
                    )
                    Scnt = psp.tile([128, 2], F32, tag="Scnt")
                    nc.tensor.matmul(Scnt[:], halfsel[:], Cc[:], start=True, stop=True)
                    ft = itp.tile([128, 2], F32, tag="ft")
                    ge = itp.tile([128, 2], mybir.dt.int8, tag="ge")
                    nge = itp.tile([128, 2], mybir.dt.int8, tag="nge")
                    nc.vector.tensor_scalar(ft[:], Scnt[:], float(num), None, OP.subtract)
                    nc.vector.tensor_scalar(ge[:], ft[:], 0.0, None, OP.is_ge)
                    nc.vector.tensor_scalar(nge[:], ft[:], 0.0, None, OP.is_lt)
                    # lo,flo <- t,ft when count>=k ; hi,fhi <- t,ft otherwise;
                    # the retained side's f halves (Illinois).
                    nc.vector.copy_predicated(lo[:], ge[:], tcur[:])
                    nc.vector.copy_predicated(hi[:], nge[:], tcur[:])
                    nc.vector.tensor_scalar(flo[:], flo[:], 0.5, None, OP.mult)
                    nc.vector.copy_predicated(flo[:], ge[:], ft[:])
                    nc.vector.tensor_scalar(fhi[:], fhi[:], 0.5, None, OP.mult)
                    nc.vector.copy_predicated(fhi[:], nge[:], ft[:])
                    if it + 1 < n_iter:
                        den = itp.tile([128, 2], F32, tag="den")
                        rec = itp.tile([128, 2], F32, tag="rec")
                        frac = itp.tile([128, 2], F32, tag="frac")
                        stp = itp.tile([128, 2], F32, tag="stp")
                        nc.vector.tensor_sub(den[:], flo[:], fhi[:])
                        nc.vector.reciprocal(rec[:], den[:])
                        nc.vector.tensor_mul(frac[:], flo[:], rec[:])
                        nc.vector.tensor_sub(stp[:], hi[:], lo[:])
                        nc.vector.tensor_mul(stp[:], frac[:], stp[:])
                        nc.vector.tensor_add(tcur[:], lo[:], stp[:])

                # ---- masked sums with final thresholds (= lo)
                nc.vector.scalar_tensor_tensor(
                    scr[:], err0[:], lo[:, 0:1], diff0[:],
                    op0=OP.is_ge, op1=OP.mult, accum_out=MD[:, 0:1],
                )
                nc.vector.scalar_tensor_tensor(
                    scr[:], err1[:], lo[:, 1:2], diff1[:],
                    op0=OP.is_ge, op1=OP.mult, accum_out=MD[:, 1:2],
                )

                if dbg_t is not None:
                    dbg = small.tile([128, 12], F32, tag="dbg")
                    nc.vector.tensor_copy(dbg[:, 0:2], mu[:])
                    nc.vector.tensor_copy(dbg[:, 2:4], sig[:])
                    nc.vector.tensor_copy(dbg[:, 4:6], t0[:])
                    nc.vector.tensor_copy(dbg[:, 6:8], lo[:])
                    nc.vector.tensor_copy(dbg[:, 8:10], SEQ[:, 0:2])
                    nc.vector.tensor_copy(dbg[:, 10:12], MD[:])
                    nc.sync.dma_start(dbg_t.ap()[:], dbg[:])

            # ---- final reduction: loss = sum over partitions of
            #      dsq0+dsq1 (base) + masked diffs
            Sfin = psp.tile([1, 4], F32, tag="Sst")
            nc.tensor.matmul(Sfin[:, 0:2], ones[:], SEQ[:, 2:4], start=True, stop=True)
            if num >= 1:
                nc.tensor.matmul(Sfin[:, 2:4], ones[:], MD[:], start=True, stop=True)
            else:
                pass
            outT = small.tile([1, 1], F32, tag="outT")
            ncols = 4 if num >= 1 else 2
            nc.vector.reduce_sum(outT[:], Sfin[:, 0:ncols], axis=mybir.AxisListType.X)
            nc.sync.dma_start(loss_t.ap()[:], outT[:])

    if split_waits:
        # CoreSim's race detector rejects the raw NOPs, so sim builds skip
        # this; the HW compile path requires it.
        _split_multi_waits(nc)
    return nc


_build_cache = {}


def _get_program(num, weight, w=W):
    key = (num, float(weight), w)
    if key not in _build_cache:
        npb = H * w
        if num >= 1:
            q = 1.0 - num / float(npb)
            a_const = NormalDist().inv_cdf(q)
            delta = 0.75 if num >= 1000 else 1.5
            n_iter = 12 if num >= 1000 else 16
        else:
            a_const, delta, n_iter = 0.0, 0.0, 0
        _build_cache[key] = build_program(num, weight, a_const, delta, n_iter, w=w)
    return _build_cache[key]


def make_consts():
    c = np.zeros((128, 225), np.float32)
    for m in range(32):           # BD4: sum groups of 4 partitions
        c[4 * m:4 * m + 4, m] = 1.0
    for m in range(64):           # PD: sum partition pairs
        c[2 * m:2 * m + 2, 32 + m] = 1.0
    c[0:64, 96:160] = 1.0         # halfsel upper-left block
    c[64:128, 160:224] = 1.0      # halfsel lower-right block
    c[:, 224] = 1.0               # ones
    return c


def make_in_maps(map0, map1, gt_density, w=W):
    gw = w * POOL
    m0 = np.ascontiguousarray(np.asarray(map0, dtype=np.float32)).reshape(B, H, w)
    m1 = np.ascontiguousarray(np.asarray(map1, dtype=np.float32)).reshape(B, H, w)
    gt = np.ascontiguousarray(np.asarray(gt_density, dtype=np.float32)).reshape(
        B, H * POOL, gw
    )
    cst = make_consts()
    in_maps = []
    for c in range(N_CORES):
        bs = slice(c * BPC, (c + 1) * BPC)
        in_maps.append(
            {
                "map0": m0[bs].reshape(BPC * H, w),
                "map1": m1[bs].reshape(BPC * H, w),
                "gt": gt[bs].reshape(BPC * H * POOL, gw),
                "consts": cst,
            }
        )
    return in_maps


def kernel(map0, map1, gt_density, process):
    p = float(process)
    weight = 1.0 * p
    noisy_ratio = 0.1 * p
    num = int(H * W * noisy_ratio)
    nc = _get_program(num, weight)
    in_maps = make_in_maps(map0, map1, gt_density)
    res = run_bass_kernel_spmd(nc, in_maps, list(range(N_CORES)))
    total = 0.0
    for c in range(N_CORES):
        total += float(res.results[c]["loss"][0, 0])
    return np.float32(total)


# revision 36
# speedup vs baseline: 1.3634x; 1.0596x over previous
"""Trainium2 Bass kernel for CHSLoss (top-k masked MSE), 8-core data parallel.

Math (per batch row, n = H*W elements, k = int(n * 0.1 * process)):
    gt   = 8x8 sum-pool of gt_density
    d_i  = map_i - gt,  err_i = |d_i|
    mask_i = err_i >= (k-th largest of err_i)
    loss += sum(d_i^2) + sum(mask_i * ((d_i - w*d_j)^2 - d_i^2))   (j != i)

Device strategy per core (2 batches/core):
  - row-pool via SWDGE accumulate-DMA (8 passes, CCE add), col-pool via a
    3D free-dim reduce -> pooled gt in SBUF; all elementwise stages on
    DVE/ACT.
  - threshold search: per-(map,batch) Gaussian-stats initial bracket
    (t0 = mu + a*sigma, a = Phi^-1(1 - k/n)), then bracketed
    Illinois-secant iterations on exact fp32 counts.  Counts use
    tensor_scalar(is_ge) with per-partition thresholds; cross-partition
    per-batch sums via a PE matmul against a half-selector matrix, which
    also broadcasts the result to all partitions.
  - partition layout: pieces of 64 row-blocks interleaved by batch, so in
    every [128, x] tile partitions 0..63 hold batch 0 and 64..127 batch 1.
  - final: per-partition accumulators -> ones-matmul column sums ->
    scalar loss per core; host sums the 8 partials.
"""
import sys

sys.path.insert(0, "/opt/trn_rl_repo")

import math
from statistics import NormalDist

import numpy as np

import concourse.bass as bass
import concourse.tile as tile
from concourse import mybir
from concourse import bass_utils
from concourse.bass_utils import run_bass_kernel_spmd

F32 = mybir.dt.float32
OP = mybir.AluOpType

# Artifact upload needs a bucket; keep traces local.
bass_utils.upload_artifacts = lambda tmpdir: f"local:{tmpdir}"


def _patched_drain_and_barrier(self, tick_clock, wait_clock):
    # This walrus build rejects >1 sync-wait on CTRL instructions ("Too many
    # sync wait commands"); split the tail-drain waits into single-wait NOPs.
    nc = self.nc
    drain_inst = nc.sync.drain()
    wait_clock.add_sem_waits(
        drain_inst.ins, tile.ScopedClock({None: tick_clock.global_clock})
    )
    si = drain_inst.ins.sync_info
    waits = list(si.on_wait) if si is not None else []
    if len(waits) > 1:
        si.on_wait = []
        id2handle = {h.num: h for h in self.sems.allocated().values()}
        for w in waits:
            nc.sync.wait_ge(id2handle[w.id], w.wait_value)
    nc.all_engine_barrier()
    popped = nc._tile_sem_poison_stack.pop()
    assert popped is self._sem_poison
    nc.clear_and_free_semaphores(list(self.sems.allocated().values()))
    nc.all_engine_barrier()


tile.TileContext._drain_and_barrier = _patched_drain_and_barrier

_NOP_CLS = None
_split_ctr = [0]


def _split_multi_waits(nc):
    """This walrus build allows at most one sync-wait per instruction; peel
    extra waits onto single-wait NOPs inserted just before, on the same
    engine."""
    global _NOP_CLS
    if _NOP_CLS is None:
        import bass_rust

        _NOP_CLS = bass_rust.InstNoOp
    import bass_rust

    for f in nc.m.functions:
        for blk in f.blocks:
            insts = blk.instructions
            out = []
            changed = False
            for ins in insts:
                si = ins.sync_info
                if si is not None and len(si.on_wait) > 1:
                    waits = list(si.on_wait)
                    for w in waits[:-1]:
                        _split_ctr[0] += 1
                        nop = _NOP_CLS(name=f"wsplit_{_split_ctr[0]}")
                        nop.engine = ins.engine
                        nop.sync_info = bass_rust.SyncInfo(
                            on_wait=[w], on_update=[]
                        )
                        out.append(nop)
                    si.on_wait = [waits[-1]]
                    changed = True
                out.append(ins)
            if changed:
                blk.instructions = out

# Problem geometry (hardcoded per spec nn_CHSLoss_75582834475514)
POOL = 8
B, H, W = 16, 192, 256  # full batch, pooled map height/width
N_CORES = 8
BPC = B // N_CORES      # batches per core = 2
NPB = H * W             # elements per batch row = 49152
PIECES = H // 64        # 3 pieces of 64 row-blocks per batch


def build_program(num, weight, a_const, delta, n_iter, w=W, debug=False,
                  split_waits=True):
    """Build the per-core Bass program.  `w` is the pooled width (reduced in
    sim tests); gt width is w*POOL."""
    gw = w * POOL
    npb = H * w
    cols = PIECES * w  # free size of full per-map tensors

    nc = bass.Bass("TRN2", target_bir_lowering=False, debug=False, num_devices=1)
    map0_t = nc.dram_tensor("map0", [BPC * H, w], F32, kind="ExternalInput")
    map1_t = nc.dram_tensor("map1", [BPC * H, w], F32, kind="ExternalInput")
    gt_t = nc.dram_tensor("gt", [BPC * H * POOL, gw], F32, kind="ExternalInput")
    consts_t = nc.dram_tensor("consts", [128, 225], F32, kind="ExternalInput")
    loss_t = nc.dram_tensor("loss", [1, 1], F32, kind="ExternalOutput")
    dbg_t = (
        nc.dram_tensor("dbg", [128, 12], F32, kind="ExternalOutput")
        if debug
        else None
    )

    with tile.TileContext(nc) as tc:
        with (
            tc.tile_pool(name="big", bufs=1) as big,
            tc.tile_pool(name="chk", bufs=6) as chp,
            tc.tile_pool(name="small", bufs=1) as small,
            tc.tile_pool(name="it", bufs=2) as itp,
            tc.tile_pool(name="q8", bufs=3, space="PSUM") as q8p,
            tc.tile_pool(name="psum", bufs=1, space="PSUM") as psp,
        ):
            # ---- constants (host-generated: partition-offset memsets are
            # not supported): cols 0:32 BD4, 32:96 PD, 96:224 halfsel,
            # 224:225 ones
            CONSTS = small.tile([128, 225], F32, tag="CONSTS")
            nc.sync.dma_start(CONSTS[:], consts_t.ap()[:])
            BD4 = CONSTS[:, 0:32]
            PD = CONSTS[:, 32:96]
            halfsel = CONSTS[:, 96:224]
            ones = CONSTS[:, 224:225]

            # ---- persistent per-element tensors [128, cols]
            m0 = big.tile([128, cols], F32, tag="m0")
            m1 = big.tile([128, cols], F32, tag="m1")
            Pg = big.tile([128, cols], F32, tag="Pg")
            d0 = big.tile([128, cols], F32, tag="d0")
            d1 = big.tile([128, cols], F32, tag="d1")
            err0 = big.tile([128, cols], F32, tag="err0")
            err1 = big.tile([128, cols], F32, tag="err1")
            dsq0 = big.tile([128, cols], F32, tag="dsq0")
            dsq1 = big.tile([128, cols], F32, tag="dsq1")
            diff0 = big.tile([128, cols], F32, tag="diff0")
            diff1 = big.tile([128, cols], F32, tag="diff1")
            scr = big.tile([128, cols], F32, tag="scr")

            # per-partition reduction accumulators:
            # SEQ cols: [sum(err0), sum(err1), sum(dsq0), sum(dsq1)]
            SEQ = small.tile([128, 4], F32, tag="SEQ")
            # MD cols: [sum(mask0*diff0), sum(mask1*diff1)]
            MD = small.tile([128, 2], F32, tag="MD")

            # map views: [2, 192, w] batch-major in DRAM
            m0v = map0_t.ap().rearrange("(b r) c -> b r c", b=BPC)
            m1v = map1_t.ap().rearrange("(b r) c -> b r c", b=BPC)
            map_dmas = []
            for x in range(PIECES):
                s = slice(x * w, (x + 1) * w)
                rsl = slice(64 * x, 64 * (x + 1))
                map_dmas.append((m0[:, s], m0v[:, rsl, :]))
                map_dmas.append((m1[:, s], m1v[:, rsl, :]))

            # ---- pooling: plain full-rate loads of [128, gw] row-chunks;
            # 4-row sums on PE (block-diagonal matmul, M=32, so PSUM write
            # bases stay 32-aligned) with 4 chunks stacked into one
            # [128, gw] PSUM tile; one 3D col-pool reduce per group; a tiny
            # PE pair-sum matmul completes the 8-row pool.
            # (Accumulate-DMA row-pooling is CCE-RMW-limited to ~166 GB/s,
            # more than 2x slower than plain loads.)
            gtr = gt_t.ap()  # [BPC*H*POOL, gw] rows
            n_chunks = BPC * H * POOL // 128  # 24
            n_groups = n_chunks // 3  # 8 groups of 3 chunks = 48 row-blocks
            Pc = [
                small.tile([48, w], F32, tag=f"Pc{_x}", name=f"Pc{_x}")
                for _x in range(n_groups)
            ]
            # Col-pool first, split across engines per chunk: GpSimd does
            # the widest halving (pair-add), DVE finishes with two strided
            # pair-adds, then PE row-pools the [128, w] col-pooled chunk
            # with a cheap N=w matmul (fp32 PE matmuls are 2-pass; keeping
            # them at N=w makes PE work negligible).
            for g in range(n_groups):
                Q8 = q8p.tile([96, w], F32, tag="Q8")
                for j in range(3):
                    jc = 3 * g + j
                    ch = chp.tile([128, gw], F32, tag="ch")
                    nc.sync.dma_start(ch[:], gtr[128 * jc:128 * (jc + 1), :])
                    chv = ch[:].rearrange("p (g two) -> p g two", two=2)
                    A = itp.tile([128, gw // 2], F32, tag="A")
                    nc.gpsimd.tensor_add(A[:], chv[:, :, 0], chv[:, :, 1])
                    Av = A[:].rearrange("p (g two) -> p g two", two=2)
                    Bt = itp.tile([128, gw // 4], F32, tag="Bt")
                    nc.vector.tensor_add(Bt[:], Av[:, :, 0], Av[:, :, 1])
                    Bv = Bt[:].rearrange("p (g two) -> p g two", two=2)
                    Cp = itp.tile([128, w], F32, tag="Cp")
                    nc.vector.tensor_add(Cp[:], Bv[:, :, 0], Bv[:, :, 1])
                    nc.tensor.matmul(
                        Q8[32 * j:32 * (j + 1), :], BD4[:], Cp[:],
                        start=True, stop=True,
                    )
                Pc4 = itp.tile([96, w], F32, tag="Pc4")
                nc.scalar.copy(Pc4[:], Q8[:])
                PS2 = psp.tile([48, w], F32, tag="PS2")
                nc.tensor.matmul(PS2[:], PD[0:96, 0:48], Pc4[:], start=True, stop=True)  # noqa: E501
                nc.scalar.copy(Pc[g][:], PS2[:])
            for dst, src in map_dmas:
                nc.sync.dma_start(dst, src)
            # group g holds rb' = 48g..48g+48 (contiguous, rb' = b*H + rb).
            # Piece x of the batch-interleaved layout needs rb' in
            # [b*H + 64x, b*H + 64x + 64) at partitions 64b..64b+64.
            for x in range(PIECES):
                s = slice(x * w, (x + 1) * w)
                for b in range(2):
                    lo_rbp = b * H + 64 * x
                    done = 0
                    while done < 64:
                        rbp = lo_rbp + done
                        g = rbp // 48
                        off = rbp % 48
                        take = min(48 - off, 64 - done)
                        nc.sync.dma_start(
                            Pg[64 * b + done:64 * b + done + take, s],
                            Pc[g][off:off + take, :],
                        )
                        done += take

            # ---- elementwise stages, per piece
            wneg = -float(weight)
            for x in range(PIECES):
                s = slice(x * w, (x + 1) * w)
                nc.vector.tensor_sub(d0[:, s], m0[:, s], Pg[:, s])
                nc.vector.tensor_sub(d1[:, s], m1[:, s], Pg[:, s])
                nc.vector.scalar_tensor_tensor(
                    err0[:, s], d0[:, s], -1.0, d0[:, s], op0=OP.mult, op1=OP.max
                )
                nc.vector.scalar_tensor_tensor(
                    err1[:, s], d1[:, s], -1.0, d1[:, s], op0=OP.mult, op1=OP.max
                )
                nc.scalar.square(dsq0[:, s], d0[:, s])
                nc.scalar.square(dsq1[:, s], d1[:, s])
                if num >= 1:
                    e0x = itp.tile([128, w], F32, tag="e0x")
                    e1x = itp.tile([128, w], F32, tag="e1x")
                    nc.vector.scalar_tensor_tensor(
                        e0x[:], d1[:, s], wneg, d0[:, s], op0=OP.mult, op1=OP.add
                    )
                    nc.vector.scalar_tensor_tensor(
                        e1x[:], d0[:, s], wneg, d1[:, s], op0=OP.mult, op1=OP.add
                    )
                    esq0x = itp.tile([128, w], F32, tag="esq0x")
                    esq1x = itp.tile([128, w], F32, tag="esq1x")
                    nc.scalar.square(esq0x[:], e0x[:])
                    nc.scalar.square(esq1x[:], e1x[:])
                    nc.vector.tensor_sub(diff0[:, s], esq0x[:], dsq0[:, s])
                    nc.vector.tensor_sub(diff1[:, s], esq1x[:], dsq1[:, s])

            # ---- per-partition sums for stats + loss base
            nc.vector.reduce_sum(SEQ[:, 0:1], err0[:], axis=mybir.AxisListType.X)
            nc.vector.reduce_sum(SEQ[:, 1:2], err1[:], axis=mybir.AxisListType.X)
            nc.vector.reduce_sum(SEQ[:, 2:3], dsq0[:], axis=mybir.AxisListType.X)
            nc.vector.reduce_sum(SEQ[:, 3:4], dsq1[:], axis=mybir.AxisListType.X)

            if num >= 1:
                # ---- stats -> initial bracket [t0 - delta, t0 + delta]
                Sst = psp.tile([128, 4], F32, tag="Sst")
                nc.tensor.matmul(Sst[:], halfsel[:], SEQ[:], start=True, stop=True)
                mu = small.tile([128, 2], F32, tag="mu")
                ex2 = small.tile([128, 2], F32, tag="ex2")
                inv_n = 1.0 / float(npb)
                nc.vector.tensor_scalar(mu[:], Sst[:, 0:2], inv_n, None, OP.mult)
                nc.vector.tensor_scalar(ex2[:], Sst[:, 2:4], inv_n, None, OP.mult)
                var = small.tile([128, 2], F32, tag="var")
                nc.vector.tensor_mul(var[:], mu[:], mu[:])
                nc.vector.tensor_sub(var[:], ex2[:], var[:])
                sig = small.tile([128, 2], F32, tag="sig")
                nc.scalar.sqrt(sig[:], var[:])
                t0 = small.tile([128, 2], F32, tag="t0")
                nc.vector.scalar_tensor_tensor(
                    t0[:], sig[:], float(a_const), mu[:], op0=OP.mult, op1=OP.add
                )
                lo = small.tile([128, 2], F32, tag="lo")
                hi = small.tile([128, 2], F32, tag="hi")
                tcur = small.tile([128, 2], F32, tag="tcur")
                flo = small.tile([128, 2], F32, tag="flo")
                fhi = small.tile([128, 2], F32, tag="fhi")
                nc.vector.tensor_scalar(lo[:], t0[:], float(delta), None, OP.subtract)
                nc.vector.tensor_scalar(hi[:], t0[:], float(delta), None, OP.add)
                nc.vector.tensor_copy(tcur[:], t0[:])
                nc.vector.memset(flo[:], float(npb - num))
                nc.vector.memset(fhi[:], float(-num))

                # ---- Illinois-secant iterations on exact counts
                for it in range(n_iter):
                    Cc = itp.tile([128, 2], F32, tag="Cc")
                    nc.vector.tensor_scalar(
                        scr[:], err0[:], tcur[:, 0:1], None, OP.is_ge, OP.add,
                        accum_out=Cc[:, 0:1],
                    )
                    nc.vector.tensor_scalar(
                        scr[:], err1[:], tcur[:, 1:2], None, OP.is_ge, OP.add,
                        accum_out=Cc[:, 1:2],
                    )
                    Scnt = psp.tile([128, 2], F32, tag="Scnt")
                    nc.tensor.matmul(Scnt[:], halfsel[:], Cc[:], start=True, stop=True)
                    ft = itp.tile([128, 2], F32, tag="ft")
                    ge = itp.tile([128, 2], mybir.dt.int8, tag="ge")
                    nge = itp.tile([128, 2], mybir.dt.int8, tag="nge")
                    nc.vector.tensor_scalar(ft[:], Scnt[:], float(num), None, OP.subtract)
                    nc.vector.tensor_scalar(ge[:], ft[:], 0.0, None, OP.is_ge)
                    nc.vector.tensor_scalar(nge[:], ft[:], 0.0, None, OP.is_lt)
                    # lo,flo <- t,ft when count>=k ; hi,fhi <- t,ft otherwise;
                    # the retained side's f halves (Illinois).
                    nc.vector.copy_predicated(lo[:], ge[:], tcur[:])
                    nc.vector.copy_predicated(hi[:], nge[:], tcur[:])
                    nc.vector.tensor_scalar(flo[:], flo[:], 0.5, None, OP.mult)
                    nc.vector.copy_predicated(flo[:], ge[:], ft[:])
                    nc.vector.tensor_scalar(fhi[:], fhi[:], 0.5, None, OP.mult)
                    nc.vector.copy_predicated(fhi[:], nge[:], ft[:])
                    if it + 1 < n_iter:
                        den = itp.tile([128, 2], F32, tag="den")
                        rec = itp.tile([128, 2], F32, tag="rec")
                        frac = itp.tile([128, 2], F32, tag="frac")
                        stp = itp.tile([128, 2], F32, tag="stp")
                        nc.vector.tensor_sub(den[:], flo[:], fhi[:])
                        nc.vector.reciprocal(rec[:], den[:])
                        nc.vector.tensor_mul(frac[:], flo[:], rec[:])
                        nc.vector.tensor_sub(stp[:], hi[:], lo[:])
                        nc.vector.tensor_mul(stp[:], frac[:], stp[:])
                        nc.vector.tensor_add(tcur[:], lo[:], stp[:])

                # ---- masked sums with final thresholds (= lo)
                nc.vector.scalar_tensor_tensor(
                    scr[:], err0[:], lo[:, 0:1], diff0[:],
                    op0=OP.is_ge, op1=OP.mult, accum_out=MD[:, 0:1],
                )
                nc.vector.scalar_tensor_tensor(
                    scr[:], err1[:], lo[:, 1:2], diff1[:],
                    op0=OP.is_ge, op1=OP.mult, accum_out=MD[:, 1:2],
                )

                if dbg_t is not None:
                    dbg = small.tile([128, 12], F32, tag="dbg")
                    nc.vector.tensor_copy(dbg[:, 0:2], mu[:])
                    nc.vector.tensor_copy(dbg[:, 2:4], sig[:])
                    nc.vector.tensor_copy(dbg[:, 4:6], t0[:])
                    nc.vector.tensor_copy(dbg[:, 6:8], lo[:])
                    nc.vector.tensor_copy(dbg[:, 8:10], SEQ[:, 0:2])
                    nc.vector.tensor_copy(dbg[:, 10:12], MD[:])
                    nc.sync.dma_start(dbg_t.ap()[:], dbg[:])

            # ---- final reduction: loss = sum over partitions of
            #      dsq0+dsq1 (base) + masked diffs
            Sfin = psp.tile([1, 4], F32, tag="Sst")
            nc.tensor.matmul(Sfin[:, 0:2], ones[:], SEQ[:, 2:4], start=True, stop=True)
            if num >= 1:
                nc.tensor.matmul(Sfin[:, 2:4], ones[:], MD[:], start=True, stop=True)
            else:
                pass
            outT = small.tile([1, 1], F32, tag="outT")
            ncols = 4 if num >= 1 else 2
            nc.vector.reduce_sum(outT[:], Sfin[:, 0:ncols], axis=mybir.AxisListType.X)
            nc.sync.dma_start(loss_t.ap()[:], outT[:])

    if split_waits:
        # CoreSim's race detector rejects the raw NOPs, so sim builds skip
        # this; the HW compile path requires it.
        _split_multi_waits(nc)
    return nc


_build_cache = {}


def _get_program(num, weight, w=W):
    key = (num, float(weight), w)
    if key not in _build_cache:
        npb = H * w
        if num >= 1:
            q = 1.0 - num / float(npb)
            a_const = NormalDist().inv_cdf(q)
            delta = 0.75 if num >= 1000 else 1.5
            n_iter = 10 if num >= 1000 else 16
        else:
            a_const, delta, n_iter = 0.0, 0.0, 0
        _build_cache[key] = build_program(num, weight, a_const, delta, n_iter, w=w)
    return _build_cache[key]


def make_consts():
    c = np.zeros((128, 225), np.float32)
    for m in range(32):           # BD4: sum groups of 4 partitions
        c[4 * m:4 * m + 4, m] = 1.0
    for m in range(64):           # PD: sum partition pairs
        c[2 * m:2 * m + 2, 32 + m] = 1.0
    c[0:64, 96:160] = 1.0         # halfsel upper-left block
    c[64:128, 160:224] = 1.0      # halfsel lower-right block
    c[:, 224] = 1.0               # ones
    return c


def make_in_maps(map0, map1, gt_density, w=W):
    gw = w * POOL
    m0 = np.ascontiguousarray(np.asarray(map0, dtype=np.float32)).reshape(B, H, w)
    m1 = np.ascontiguousarray(np.asarray(map1, dtype=np.float32)).reshape(B, H, w)
    gt = np.ascontiguousarray(np.asarray(gt_density, dtype=np.float32)).reshape(
        B, H * POOL, gw
    )
    cst = make_consts()
    in_maps = []
    for c in range(N_CORES):
        bs = slice(c * BPC, (c + 1) * BPC)
        in_maps.append(
            {
                "map0": m0[bs].reshape(BPC * H, w),
                "map1": m1[bs].reshape(BPC * H, w),
                "gt": gt[bs].reshape(BPC * H * POOL, gw),
                "consts": cst,
            }
        )
    return in_maps


def kernel(map0, map1, gt_density, process):
    p = float(process)
    weight = 1.0 * p
    noisy_ratio = 0.1 * p
    num = int(H * W * noisy_ratio)
    nc = _get_program(num, weight)
    in_maps = make_in_maps(map0, map1, gt_density)
    res = run_bass_kernel_spmd(nc, in_maps, list(range(N_CORES)))
    total = 0.0
    for c in range(N_CORES):
        total += float(res.results[c]["loss"][0, 0])
    return np.float32(total)
